# revision 3
# baseline (speedup 1.0000x reference)
"""Bayesian transformer block on 8 trn2 cores — bf16-resident version.

Sharding: core c -> batch b=c//2, half h=c%2. Each core's xf is pre-rolled
by 256*h inside every 512-row group, so its 256 query rows per chunk are
always rolled rows [0,256) of group p — the program is identical on all
cores. K^T, V, and transposed-x tiles are bf16 and fully SBUF-resident
(no DRAM spill); weights are perturbed in f32 then stored transposed in
bf16. wo/w1/w2 mu/ls/eps stream on the Pool engine's DMA queue in
parallel with SP's wk/wv/xf stream, and their build blocks are spliced
into the KV loop so PE covers the weight-DMA latency. Scores use
transposed layout S^T[k,q]; row sums via an all-ones stationary matmul
into a dedicated PSUM bank; FFN accumulates ff2 in a single bank,
q-halves sequentially, from stored f1 tiles.
"""
import sys, os

for _p in ("/opt/trn_rl_repo", "/root/.axon_site/_ro/trn_rl_repo"):
    if os.path.isdir(_p) and _p not in sys.path:
        sys.path.insert(0, _p)

import numpy as np
from contextlib import ExitStack

import concourse.bass as bass
import concourse.bacc as bacc
import concourse.mybir as mybir
import concourse.tile as tile
from concourse.bass_utils import run_bass_kernel_spmd
from concourse.masks import make_identity

F32 = mybir.dt.float32
BF16 = mybir.dt.bfloat16
AF = mybir.ActivationFunctionType
OP = mybir.AluOpType

DIM = 512
HID = 2048
BS, SLEN = 4, 4096
NCHUNK = 8
QC = 256
NQROWS = NCHUNK * QC
NG = SLEN // 512
INV_SQRT_D = float(1.0 / np.sqrt(DIM))

_CACHE = {}


def _build_nc():
    nc = bacc.Bacc("TRN2", target_bir_lowering=False, debug=False, num_devices=8,
                   dynamic_dma_scratch_size=2048)

    xf = nc.dram_tensor("xf", [SLEN, DIM], F32, kind="ExternalInput").ap()
    cmask = nc.dram_tensor("cmask", [4, 128, QC], F32, kind="ExternalInput").ap()
    wio = {}
    for w, (o, i) in (("wk", (DIM, DIM)), ("wv", (DIM, DIM)), ("wo", (DIM, DIM)),
                      ("w1", (HID, DIM)), ("w2", (DIM, HID))):
        for sfx in ("mu", "ls", "eps"):
            wio[f"{w}_{sfx}"] = nc.dram_tensor(f"{w}_{sfx}", [o, i], F32,
                                               kind="ExternalInput").ap()
    out = nc.dram_tensor("out", [NQROWS, DIM], F32, kind="ExternalOutput").ap()

    with tile.TileContext(nc) as tc:
      with ExitStack() as ctx:
        const = ctx.enter_context(tc.tile_pool(name="const", bufs=1))
        wres = ctx.enter_context(tc.tile_pool(name="wres", bufs=1))
        kvcache = ctx.enter_context(tc.tile_pool(name="kvcache", bufs=1))
        # shared transpose/score PSUM pool, alive for the whole kernel
        psT = ctx.enter_context(tc.tile_pool(name="psT", bufs=2, space="PSUM"))

        ident = const.tile([128, 128], F32, tag="ident")
        make_identity(nc, ident[:])
        ident_b = const.tile([128, 128], BF16, tag="ident_b")
        nc.vector.tensor_copy(ident_b[:], ident[:])
        ones32 = const.tile([128, 128], F32, tag="ones32")
        nc.gpsimd.memset(ones32[:], 1.0)
        ones = const.tile([128, 128], BF16, tag="ones")
        nc.vector.tensor_copy(ones[:], ones32[:])
        cm = const.tile([128, 4, QC], F32, tag="cm")

        def build_blocks(w, o_dim, i_dim, dst_pool, stage, dmaq, prod_eng,
                         add_eng=None):
            """Perturb W = mu + exp(ls)*eps and store W^T as bf16 tiles.
            Returns (tiles, generator); each generator step emits one
            [128 x IC] block: 3 DMAs on `dmaq`, exp on ACT, mult on
            `prod_eng`, add on `add_eng`, transposes into psT + copies."""
            wt = [dst_pool.tile([128, o_dim], BF16, tag=f"{w}T{i}", name=f"{w}T{i}")
                  for i in range(i_dim // 128)]
            mu_r = wio[f"{w}_mu"].rearrange("(a p) i -> a p i", p=128)
            ls_r = wio[f"{w}_ls"].rearrange("(a p) i -> a p i", p=128)
            ep_r = wio[f"{w}_eps"].rearrange("(a p) i -> a p i", p=128)
            IC = min(i_dim, 512)

            def gen():
                for a in range(o_dim // 128):
                    for cb in range(i_dim // IC):
                        mu = stage.tile([128, IC], F32, tag="mu")
                        ls = stage.tile([128, IC], F32, tag="ls")
                        ep = stage.tile([128, IC], F32, tag="ep")
                        dmaq.dma_start(mu[:], mu_r[a][:, bass.ts(cb, IC)])
                        dmaq.dma_start(ls[:], ls_r[a][:, bass.ts(cb, IC)])
                        dmaq.dma_start(ep[:], ep_r[a][:, bass.ts(cb, IC)])
                        els = stage.tile([128, IC], F32, tag="els")
                        nc.scalar.activation(els[:], ls[:], AF.Exp)
                        prod = stage.tile([128, IC], F32, tag="prod")
                        prod_eng.tensor_tensor(prod[:], els[:], ep[:], op=OP.mult)
                        wnat = stage.tile([128, IC], BF16, tag="wnat")
                        (add_eng or nc.vector).tensor_tensor(wnat[:], prod[:],
                                                             mu[:], op=OP.add)
                        for ii in range(IC // 128):
                            i = cb * (IC // 128) + ii
                            ps = psT.tile([128, 128], BF16, tag="sT")
                            nc.tensor.transpose(ps[:], wnat[:, bass.ts(ii, 128)],
                                                ident_b[:])
                            if i % 2 == 0:
                                nc.vector.tensor_copy(wt[i][:, bass.ts(a, 128)], ps[:])
                            else:
                                nc.scalar.copy(wt[i][:, bass.ts(a, 128)], ps[:])
                        yield

            return wt, gen()

        # ---- K^T / V / x^T over all 4096 (rolled) keys, all bf16-resident ----
        xf_r = xf.rearrange("(g j p) d -> g j p d", j=4, p=128)

        ktc = [[kvcache.tile([128, 512], BF16, tag=f"ktc{g}_{i}", name=f"ktc{g}_{i}")
                for i in range(4)] for g in range(NG)]
        vc = [[kvcache.tile([128, 512], BF16, tag=f"vc{g}_{j}", name=f"vc{g}_{j}")
               for j in range(4)] for g in range(NG)]
        xfT = [[kvcache.tile([128, 512], BF16, tag=f"xfT{g}_{i}", name=f"xfT{g}_{i}")
                for i in range(4)] for g in range(NG)]

        stk = ctx.enter_context(ExitStack())
        if True:
            stageB = stk.enter_context(tc.tile_pool(name="stgB", bufs=3))
            # wo/w2 are built inside the KV loop via generator splicing;
            # w1 is built interleaved into chunk 7's attention units, so the
            # single DMA queue is balanced between the two regions.
            woT, genO = build_blocks("wo", DIM, DIM, wres, stageB, nc.sync,
                                     nc.vector)
            w2T, gen2 = build_blocks("w2", DIM, HID, wres, stageB, nc.sync,
                                     nc.vector)
            # w1's elementwise work runs on Pool, keeping DVE clear for the
            # attention-region work it interleaves with
            w1T, gen1 = build_blocks("w1", HID, DIM, wres, stageB, nc.sync,
                                     nc.gpsimd, nc.gpsimd)

            def _chain(*gens):
                for g_ in gens:
                    yield from g_

            rem = _chain(genO, gen2)

            with ExitStack() as stkB:
                wkv = stkB.enter_context(tc.tile_pool(name="wkv", bufs=1))
                wkT, genK = build_blocks("wk", DIM, DIM, wkv, stageB, nc.sync,
                                         nc.vector)
                wvT, genV = build_blocks("wv", DIM, DIM, wkv, stageB, nc.sync,
                                         nc.vector)
                for _ in genK:
                    next(genV, None)
                for _ in genV:
                    pass
                stage = stkB.enter_context(tc.tile_pool(name="stg_x", bufs=3))
                psb = stkB.enter_context(tc.tile_pool(name="psB", bufs=2,
                                                      space="PSUM"))
                for g in range(NG):
                    for j in range(4):
                        xt = stage.tile([128, DIM], F32, tag="xrow")
                        nc.sync.dma_start(xt[:], xf_r[g, j])
                        for i in range(4):
                            ps = psT.tile([128, 128], F32, tag="sT")
                            nc.tensor.transpose(ps[:], xt[:, bass.ts(i, 128)],
                                                ident[:])
                            if g >= NG - 2 or (j + i) % 2 == 0:
                                nc.vector.tensor_copy(
                                    xfT[g][i][:, bass.ts(j, 128)], ps[:])
                            else:
                                nc.scalar.copy(xfT[g][i][:, bass.ts(j, 128)], ps[:])
                    for o in range(4):
                        ps = psb.tile([128, 512], F32, tag="kps")
                        for i in range(4):
                            nc.tensor.matmul(ps[:], wkT[i][:, bass.ts(o, 128)],
                                             xfT[g][i][:], start=(i == 0),
                                             stop=(i == 3))
                        if g >= NG - 2 or o % 2 == 0:
                            nc.vector.tensor_copy(ktc[g][o][:], ps[:])
                        else:
                            nc.scalar.copy(ktc[g][o][:], ps[:])
                    for j in range(4):
                        ps = psb.tile([128, 512], F32, tag="vps")
                        for i in range(4):
                            nc.tensor.matmul(ps[:], xfT[g][i][:, bass.ts(j, 128)],
                                             wvT[i][:], start=(i == 0), stop=(i == 3))
                        if g >= NG - 2 or j % 2 == 0:
                            nc.vector.tensor_copy(vc[g][j][:], ps[:])
                        else:
                            nc.scalar.copy(vc[g][j][:], ps[:])
                    for _ in range(3 if g < 4 else 2):
                        if next(rem, None) is None:
                            break
            # 20 splices >= wo+w2's 20 blocks; no-op guard
            for _ in rem:
                pass
            nc.sync.dma_start(cm[:], cmask.rearrange("j p q -> p j q"))

        # ---- per-chunk attention + FFN (descending: dense work first) ----
        slot = ctx.enter_context(tc.tile_pool(name="slot", bufs=2))
        s1 = ctx.enter_context(tc.tile_pool(name="s1", bufs=1))
        pt_pool = ctx.enter_context(tc.tile_pool(name="pt", bufs=3))
        ff_pool = ctx.enter_context(tc.tile_pool(name="ff", bufs=2))
        # PSUM (8 banks): psT 2 (transposes+scores+proj) + psH 2 (packed AV)
        # + psR 1 (s_rep) + psW 2 (ffn1) + psE 1 (ff2) = 8
        psH = ctx.enter_context(tc.tile_pool(name="psH", bufs=1, space="PSUM"))
        psR = ctx.enter_context(tc.tile_pool(name="psR", bufs=1, space="PSUM"))
        psW = ctx.enter_context(tc.tile_pool(name="psW", bufs=2, space="PSUM"))
        psE = ctx.enter_context(tc.tile_pool(name="psE", bufs=1, space="PSUM"))

        out_r = out.rearrange("(p jj q) d -> p jj q d", jj=2, q=128)

        def make_scores(p, xqT):
            def emit_scores(g, j):
                ps = psT.tile([128, QC], F32, tag="sT")
                for i in range(4):
                    nc.tensor.matmul(ps[:], ktc[g][i][:, bass.ts(j, 128)],
                                     xqT[i], start=(i == 0), stop=(i == 3))
                pt = pt_pool.tile([128, QC], BF16, tag="pt", bufs=6)
                if g == p:
                    pe = pt_pool.tile([128, QC], BF16, tag="pe", bufs=4)
                    nc.scalar.activation(pe[:], ps[:], AF.Exp, scale=INV_SQRT_D)
                    nc.vector.tensor_tensor(pt[:], pe[:], cm[:, j, :], op=OP.mult)
                else:
                    nc.scalar.activation(pt[:], ps[:], AF.Exp, scale=INV_SQRT_D)
                return pt
            return emit_scores

        def make_tail(p, xqT, hT, s_rep):
            def emit_tail():
                r_bc = slot.tile([128, QC], F32, tag="r_bc")
                nc.vector.reciprocal(r_bc[:], s_rep[:])
                h_nrm = [s1.tile([128, QC], BF16, tag=f"hn{i}", name=f"hn{i}",
                                 bufs=2) for i in range(4)]
                for i in range(4):
                    nc.vector.tensor_tensor(h_nrm[i][:], hT[i], r_bc[:], op=OP.mult)

                h_resT = [s1.tile([128, QC], BF16, tag=f"hrT{o}", name=f"hrT{o}",
                                  bufs=2) for o in range(4)]
                for o in range(4):
                    ps = psT.tile([128, QC], F32, tag="sT")
                    for i in range(4):
                        nc.tensor.matmul(ps[:], woT[i][:, bass.ts(o, 128)],
                                         h_nrm[i][:], start=(i == 0), stop=(i == 3))
                    nc.vector.tensor_tensor(h_resT[o][:], ps[:], xqT[o], op=OP.add)
                h_resN = [slot.tile([128, DIM], BF16, tag=f"hrN{jj}",
                                    name=f"hrN{jj}", bufs=2) for jj in range(2)]
                for jj in range(2):
                    for o in range(4):
                        tp = psT.tile([128, 128], BF16, tag="sT", name="tph")
                        nc.tensor.transpose(tp[:], h_resT[o][:, bass.ts(jj, 128)],
                                            ident_b[:])
                        if o % 2 == 0:
                            nc.vector.tensor_copy(h_resN[jj][:, bass.ts(o, 128)],
                                                  tp[:])
                        else:
                            nc.scalar.copy(h_resN[jj][:, bass.ts(o, 128)], tp[:])

                f1 = [ff_pool.tile([128, QC], BF16, tag=f"f1_{hh}", name=f"f1_{hh}")
                      for hh in range(HID // 128)]
                for hh in range(HID // 128):
                    ps = psW.tile([128, QC], F32, tag="wf")
                    for i in range(4):
                        nc.tensor.matmul(ps[:], w1T[i][:, bass.ts(hh, 128)],
                                         h_resT[i][:], start=(i == 0), stop=(i == 3))
                    nc.scalar.activation(f1[hh][:], ps[:], AF.Relu)
                for jj in range(2):
                    # jj=1 rotates through psW (free by then); psR holds only
                    # s_rep so the next chunk's accumulation never waits on
                    # the FFN drain
                    if jj == 0:
                        ff2 = psE.tile([128, DIM], F32, tag="ff2", name="ff2a")
                    else:
                        ff2 = psW.tile([128, DIM], F32, tag="wf", name="ff2b")
                    for hh in range(HID // 128):
                        nc.tensor.matmul(ff2[:], f1[hh][:, bass.ts(jj, 128)],
                                         w2T[hh][:], start=(hh == 0),
                                         stop=(hh == HID // 128 - 1))
                    ot = slot.tile([128, DIM], F32, tag=f"ot{jj}")
                    nc.vector.tensor_tensor(ot[:], ff2[:], h_resN[jj][:], op=OP.add)
                    nc.sync.dma_start(out_r[p, jj], ot[:])
            return emit_tail

        # cross-chunk software pipeline: the first PEEL score units of each
        # chunk are emitted before the PREVIOUS chunk's tail, so PE has score
        # work covering the reciprocal/h_nrm latency; AV then runs PEEL
        # units behind scores for the rest of the chunk.
        PEEL = 4
        prev_tail = None
        for p in range(NCHUNK - 1, -1, -1):
            xqT = [xfT[p][i][:, 0:QC] for i in range(4)]
            hTA = psH.tile([128, 2 * QC], F32, tag="hTA", name="hTA")
            hTB = psH.tile([128, 2 * QC], F32, tag="hTB", name="hTB")
            hT = [hTA[:, bass.ts(0, QC)], hTA[:, bass.ts(1, QC)],
                  hTB[:, bass.ts(0, QC)], hTB[:, bass.ts(1, QC)]]
            s_rep = psR.tile([128, QC], F32, tag="srep")
            units = [(g, j) for g in range(p + 1) for j in range(4)]
            nu = len(units)
            emit_scores = make_scores(p, xqT)

            def emit_av(u, pt):
                g, j = units[u]
                first_av = (u == 0)
                last_av = (u == nu - 1)
                for i in range(4):
                    # start=True clears the whole PSUM bank, so only the
                    # first chain per packed bank may set it; the second
                    # chain writes into cleared has_written bits.
                    nc.tensor.matmul(hT[i], vc[g][j][:, bass.ts(i, 128)], pt[:],
                                     start=first_av and (i % 2 == 0),
                                     stop=last_av, skip_group_check=True)
                nc.tensor.matmul(s_rep[:], ones[:], pt[:],
                                 start=first_av, stop=last_av)

            S = min(PEEL, nu)
            pts = [None] * nu
            for u in range(S):
                pts[u] = emit_scores(*units[u])
            if prev_tail is not None:
                prev_tail()
            for u in range(S, nu):
                pts[u] = emit_scores(*units[u])
                emit_av(u - S, pts[u - S])
                pts[u - S] = None
                if p == NCHUNK - 1:
                    # build one w1 block per unit while the DMA queue is idle
                    next(gen1, None)
            for u in range(nu - S, nu):
                emit_av(u, pts[u])
                pts[u] = None
            prev_tail = make_tail(p, xqT, hT, s_rep)
        prev_tail()
        for _ in gen1:
            pass

    nc.compile()
    return nc


def _shard_inputs(inputs):
    x = np.ascontiguousarray(inputs["x"], dtype=np.float32)
    kr = np.arange(128)[:, None]
    qr = np.arange(QC)[None, :]
    mA = (qr >= kr).astype(np.float32)
    mB = (qr >= 128 + kr).astype(np.float32)
    in_maps = []
    for c in range(8):
        b, h = c // 2, c % 2
        xb = x[b]
        if h:
            # roll each 512-row group by 256 so this core's query rows are
            # always rolled rows [0,256) of the group
            xb = np.ascontiguousarray(
                xb.reshape(NCHUNK, 2, 256, DIM)[:, ::-1].reshape(SLEN, DIM))
        cmsk = np.stack([mA, mB, np.full_like(mA, h), np.full_like(mA, h)])
        m = {"xf": np.ascontiguousarray(xb), "cmask": np.ascontiguousarray(cmsk)}
        for k, v in inputs.items():
            if k not in ("x", "mask"):
                m[k] = np.ascontiguousarray(v, dtype=np.float32)
        in_maps.append(m)
    return in_maps


def kernel(**inputs):
    if "nc" not in _CACHE:
        _CACHE["nc"] = _build_nc()
    nc = _CACHE["nc"]
    in_maps = _shard_inputs(inputs)
    res = run_bass_kernel_spmd(nc, in_maps, core_ids=list(range(8)))
    out = np.empty((BS, SLEN, DIM), dtype=np.float32)
    for c in range(8):
        b, h = c // 2, c % 2
        o = res.results[c]["out"].reshape(NCHUNK, QC, DIM)
        out.reshape(BS, NCHUNK, 512, DIM)[b, :, QC * h:QC * h + QC, :] = o
    return out


# revision 4
# speedup vs baseline: 4.0405x; 4.0405x over previous
"""Bayesian transformer block on 8 trn2 cores — bf16-resident version.

Sharding: core c -> batch b=c//2, half h=c%2. Each core's xf is pre-rolled
by 256*h inside every 512-row group, so its 256 query rows per chunk are
always rolled rows [0,256) of group p — the program is identical on all
cores. K^T, V, and transposed-x tiles are bf16 and fully SBUF-resident
(no DRAM spill); weights are perturbed in f32 then stored transposed in
bf16. wo/w1/w2 mu/ls/eps stream on the Pool engine's DMA queue in
parallel with SP's wk/wv/xf stream, and their build blocks are spliced
into the KV loop so PE covers the weight-DMA latency. Scores use
transposed layout S^T[k,q]; row sums via an all-ones stationary matmul
into a dedicated PSUM bank; FFN accumulates ff2 in a single bank,
q-halves sequentially, from stored f1 tiles.
"""
import sys, os

for _p in ("/opt/trn_rl_repo", "/root/.axon_site/_ro/trn_rl_repo"):
    if os.path.isdir(_p) and _p not in sys.path:
        sys.path.insert(0, _p)

import numpy as np
from contextlib import ExitStack

import concourse.bass as bass
import concourse.bacc as bacc
import concourse.mybir as mybir
import concourse.tile as tile
from concourse.bass_utils import run_bass_kernel_spmd
from concourse.masks import make_identity

F32 = mybir.dt.float32
BF16 = mybir.dt.bfloat16
AF = mybir.ActivationFunctionType
OP = mybir.AluOpType

DIM = 512
HID = 2048
BS, SLEN = 4, 4096
NCHUNK = 8
QC = 256
NQROWS = NCHUNK * QC
NG = SLEN // 512
INV_SQRT_D = float(1.0 / np.sqrt(DIM))

_CACHE = {}


def _build_nc():
    nc = bacc.Bacc("TRN2", target_bir_lowering=False, debug=False, num_devices=8,
                   dynamic_dma_scratch_size=2048)

    xf = nc.dram_tensor("xf", [SLEN, DIM], F32, kind="ExternalInput").ap()
    cmask = nc.dram_tensor("cmask", [4, 128, QC], F32, kind="ExternalInput").ap()
    wio = {}
    for w, (o, i) in (("wk", (DIM, DIM)), ("wv", (DIM, DIM)), ("wo", (DIM, DIM)),
                      ("w1", (HID, DIM)), ("w2", (DIM, HID))):
        for sfx in ("mu", "ls", "eps"):
            wio[f"{w}_{sfx}"] = nc.dram_tensor(f"{w}_{sfx}", [o, i], F32,
                                               kind="ExternalInput").ap()
    out = nc.dram_tensor("out", [NQROWS, DIM], F32, kind="ExternalOutput").ap()

    with tile.TileContext(nc) as tc:
      with ExitStack() as ctx:
        const = ctx.enter_context(tc.tile_pool(name="const", bufs=1))
        wres = ctx.enter_context(tc.tile_pool(name="wres", bufs=1))
        kvcache = ctx.enter_context(tc.tile_pool(name="kvcache", bufs=1))
        # shared transpose/score PSUM pool, alive for the whole kernel
        psT = ctx.enter_context(tc.tile_pool(name="psT", bufs=2, space="PSUM"))

        ident = const.tile([128, 128], F32, tag="ident")
        make_identity(nc, ident[:])
        ident_b = const.tile([128, 128], BF16, tag="ident_b")
        nc.vector.tensor_copy(ident_b[:], ident[:])
        ones32 = const.tile([128, 128], F32, tag="ones32")
        nc.gpsimd.memset(ones32[:], 1.0)
        ones = const.tile([128, 128], BF16, tag="ones")
        nc.vector.tensor_copy(ones[:], ones32[:])
        cm = const.tile([128, 4, QC], F32, tag="cm")

        PSX = [None]

        def build_blocks(w, o_dim, i_dim, dst_pool, stage, dmaq, prod_eng,
                         add_eng=None, pst=None):
            """Perturb W = mu + exp(ls)*eps and store W^T as bf16 tiles.
            Returns (tiles, generator); each generator step emits one
            [128 x IC] block: 3 DMAs on `dmaq`, exp on ACT, mult on
            `prod_eng`, add on `add_eng`, transposes into psT + copies."""
            wt = [dst_pool.tile([128, o_dim], BF16, tag=f"{w}T{i}", name=f"{w}T{i}")
                  for i in range(i_dim // 128)]
            mu_r = wio[f"{w}_mu"].rearrange("(a p) i -> a p i", p=128)
            ls_r = wio[f"{w}_ls"].rearrange("(a p) i -> a p i", p=128)
            ep_r = wio[f"{w}_eps"].rearrange("(a p) i -> a p i", p=128)
            IC = min(i_dim, 512)

            def gen():
                for a in range(o_dim // 128):
                    for cb in range(i_dim // IC):
                        mu = stage.tile([128, IC], F32, tag="mu")
                        ls = stage.tile([128, IC], F32, tag="ls")
                        ep = stage.tile([128, IC], F32, tag="ep")
                        dmaq.dma_start(mu[:], mu_r[a][:, bass.ts(cb, IC)])
                        dmaq.dma_start(ls[:], ls_r[a][:, bass.ts(cb, IC)])
                        dmaq.dma_start(ep[:], ep_r[a][:, bass.ts(cb, IC)])
                        els = stage.tile([128, IC], F32, tag="els")
                        nc.scalar.activation(els[:], ls[:], AF.Exp)
                        prod = stage.tile([128, IC], F32, tag="prod")
                        prod_eng.tensor_tensor(prod[:], els[:], ep[:], op=OP.mult)
                        wnat = stage.tile([128, IC], BF16, tag="wnat")
                        (add_eng or nc.vector).tensor_tensor(wnat[:], prod[:],
                                                             mu[:], op=OP.add)
                        for ii in range(IC // 128):
                            i = cb * (IC // 128) + ii
                            pool = pst() if pst else psT
                            ps = pool.tile([128, 128], BF16, tag="sT")
                            nc.tensor.transpose(ps[:], wnat[:, bass.ts(ii, 128)],
                                                ident_b[:])
                            if i % 2 == 0:
                                nc.vector.tensor_copy(wt[i][:, bass.ts(a, 128)], ps[:])
                            else:
                                nc.scalar.copy(wt[i][:, bass.ts(a, 128)], ps[:])
                        yield

            return wt, gen()

        # ---- K^T / V / x^T over all 4096 (rolled) keys, all bf16-resident ----
        xf_r = xf.rearrange("(g j p) d -> g j p d", j=4, p=128)

        ktc = [[kvcache.tile([128, 512], BF16, tag=f"ktc{g}_{i}", name=f"ktc{g}_{i}")
                for i in range(4)] for g in range(NG)]
        vc = [[kvcache.tile([128, 512], BF16, tag=f"vc{g}_{j}", name=f"vc{g}_{j}")
               for j in range(4)] for g in range(NG)]
        xfT = [[kvcache.tile([128, 512], BF16, tag=f"xfT{g}_{i}", name=f"xfT{g}_{i}")
                for i in range(4)] for g in range(NG)]

        stk = ctx.enter_context(ExitStack())
        if True:
            stageB = stk.enter_context(tc.tile_pool(name="stgB", bufs=3))
            # wo/w2 are built inside the KV loop via generator splicing;
            # w1 is built interleaved into chunk 7's attention units, so the
            # single DMA queue is balanced between the two regions.
            woT, genO = build_blocks("wo", DIM, DIM, wres, stageB, nc.sync,
                                     nc.vector, pst=lambda: PSX[0])
            w2T, gen2 = build_blocks("w2", DIM, HID, wres, stageB, nc.sync,
                                     nc.vector, pst=lambda: PSX[0])
            # w1's elementwise work runs on Pool, keeping DVE clear for the
            # attention-region work it interleaves with
            w1T, gen1 = build_blocks("w1", HID, DIM, wres, stageB, nc.sync,
                                     nc.gpsimd, nc.gpsimd)

            def _chain(*gens):
                for g_ in gens:
                    yield from g_

            rem = _chain(genO, gen2)

            with ExitStack() as stkB:
                psX = stkB.enter_context(tc.tile_pool(name="psX", bufs=4,
                                                      space="PSUM"))
                PSX[0] = psX
                wkv = stkB.enter_context(tc.tile_pool(name="wkv", bufs=1))
                wkT, genK = build_blocks("wk", DIM, DIM, wkv, stageB, nc.sync,
                                         nc.vector, pst=lambda: PSX[0])
                wvT, genV = build_blocks("wv", DIM, DIM, wkv, stageB, nc.sync,
                                         nc.vector, pst=lambda: PSX[0])
                for _ in genK:
                    next(genV, None)
                for _ in genV:
                    pass
                stage = stkB.enter_context(tc.tile_pool(name="stg_x", bufs=3))
                psb = stkB.enter_context(tc.tile_pool(name="psB", bufs=2,
                                                      space="PSUM"))
                for g in range(NG):
                    for j in range(4):
                        xt = stage.tile([128, DIM], F32, tag="xrow")
                        nc.sync.dma_start(xt[:], xf_r[g, j])
                        for i in range(4):
                            ps = psX.tile([128, 128], F32, tag="sT")
                            nc.tensor.transpose(ps[:], xt[:, bass.ts(i, 128)],
                                                ident[:])
                            if g >= NG - 2 or (j + i) % 2 == 0:
                                nc.vector.tensor_copy(
                                    xfT[g][i][:, bass.ts(j, 128)], ps[:])
                            else:
                                nc.scalar.copy(xfT[g][i][:, bass.ts(j, 128)], ps[:])
                    for o in range(4):
                        ps = psb.tile([128, 512], F32, tag="kps")
                        for i in range(4):
                            nc.tensor.matmul(ps[:], wkT[i][:, bass.ts(o, 128)],
                                             xfT[g][i][:], start=(i == 0),
                                             stop=(i == 3))
                        if g >= NG - 2 or o % 2 == 0:
                            nc.vector.tensor_copy(ktc[g][o][:], ps[:])
                        else:
                            nc.scalar.copy(ktc[g][o][:], ps[:])
                    for j in range(4):
                        ps = psb.tile([128, 512], F32, tag="kps", name="vps")
                        for i in range(4):
                            nc.tensor.matmul(ps[:], xfT[g][i][:, bass.ts(j, 128)],
                                             wvT[i][:], start=(i == 0), stop=(i == 3))
                        if g >= NG - 2 or j % 2 == 0:
                            nc.vector.tensor_copy(vc[g][j][:], ps[:])
                        else:
                            nc.scalar.copy(vc[g][j][:], ps[:])
                    for _ in range(3 if g < 4 else 2):
                        if next(rem, None) is None:
                            break
                # 20 splices >= wo+w2's 20 blocks; no-op guard
                for _ in rem:
                    pass
            PSX[0] = None
            nc.sync.dma_start(cm[:], cmask.rearrange("j p q -> p j q"))

        # ---- per-chunk attention + FFN (descending: dense work first) ----
        slot = ctx.enter_context(tc.tile_pool(name="slot", bufs=2))
        s1 = ctx.enter_context(tc.tile_pool(name="s1", bufs=1))
        pt_pool = ctx.enter_context(tc.tile_pool(name="pt", bufs=3))
        ff_pool = ctx.enter_context(tc.tile_pool(name="ff", bufs=2))
        # PSUM (8 banks): psT 2 (transposes+scores+proj) + psH 2 (packed AV)
        # + psR 1 (s_rep) + psW 2 (ffn1) + psE 1 (ff2) = 8
        psH = ctx.enter_context(tc.tile_pool(name="psH", bufs=1, space="PSUM"))
        psR = ctx.enter_context(tc.tile_pool(name="psR", bufs=1, space="PSUM"))
        psW = ctx.enter_context(tc.tile_pool(name="psW", bufs=2, space="PSUM"))
        psE = ctx.enter_context(tc.tile_pool(name="psE", bufs=1, space="PSUM"))

        out_r = out.rearrange("(p jj q) d -> p jj q d", jj=2, q=128)

        def make_scores(p, xqT):
            def emit_scores(g, j):
                ps = psT.tile([128, QC], F32, tag="sT")
                for i in range(4):
                    nc.tensor.matmul(ps[:], ktc[g][i][:, bass.ts(j, 128)],
                                     xqT[i], start=(i == 0), stop=(i == 3))
                pt = pt_pool.tile([128, QC], BF16, tag="pt", bufs=6)
                if g == p:
                    pe = pt_pool.tile([128, QC], BF16, tag="pe", bufs=4)
                    nc.scalar.activation(pe[:], ps[:], AF.Exp, scale=INV_SQRT_D)
                    nc.vector.tensor_tensor(pt[:], pe[:], cm[:, j, :], op=OP.mult)
                else:
                    nc.scalar.activation(pt[:], ps[:], AF.Exp, scale=INV_SQRT_D)
                return pt
            return emit_scores

        def make_tail(p, xqT, hT, s_rep):
            def emit_tail():
                r_bc = slot.tile([128, QC], F32, tag="r_bc")
                nc.vector.reciprocal(r_bc[:], s_rep[:])
                h_nrm = [s1.tile([128, QC], BF16, tag=f"hn{i}", name=f"hn{i}",
                                 bufs=2) for i in range(4)]
                for i in range(4):
                    nc.vector.tensor_tensor(h_nrm[i][:], hT[i], r_bc[:], op=OP.mult)

                h_resT = [s1.tile([128, QC], BF16, tag=f"hrT{o}", name=f"hrT{o}",
                                  bufs=2) for o in range(4)]
                for o in range(4):
                    ps = psT.tile([128, QC], F32, tag="sT")
                    for i in range(4):
                        nc.tensor.matmul(ps[:], woT[i][:, bass.ts(o, 128)],
                                         h_nrm[i][:], start=(i == 0), stop=(i == 3))
                    nc.vector.tensor_tensor(h_resT[o][:], ps[:], xqT[o], op=OP.add)
                h_resN = [slot.tile([128, DIM], BF16, tag=f"hrN{jj}",
                                    name=f"hrN{jj}", bufs=2) for jj in range(2)]
                for jj in range(2):
                    for o in range(4):
                        tp = psT.tile([128, 128], BF16, tag="sT", name="tph")
                        nc.tensor.transpose(tp[:], h_resT[o][:, bass.ts(jj, 128)],
                                            ident_b[:])
                        if o % 2 == 0:
                            nc.vector.tensor_copy(h_resN[jj][:, bass.ts(o, 128)],
                                                  tp[:])
                        else:
                            nc.scalar.copy(h_resN[jj][:, bass.ts(o, 128)], tp[:])

                f1 = [ff_pool.tile([128, QC], BF16, tag=f"f1_{hh}", name=f"f1_{hh}")
                      for hh in range(HID // 128)]
                for hh in range(HID // 128):
                    ps = psW.tile([128, QC], F32, tag="wf")
                    for i in range(4):
                        nc.tensor.matmul(ps[:], w1T[i][:, bass.ts(hh, 128)],
                                         h_resT[i][:], start=(i == 0), stop=(i == 3))
                    nc.scalar.activation(f1[hh][:], ps[:], AF.Relu)
                for jj in range(2):
                    # jj=1 rotates through psW (free by then); psR holds only
                    # s_rep so the next chunk's accumulation never waits on
                    # the FFN drain
                    if jj == 0:
                        ff2 = psE.tile([128, DIM], F32, tag="ff2", name="ff2a")
                    else:
                        ff2 = psW.tile([128, DIM], F32, tag="wf", name="ff2b")
                    for hh in range(HID // 128):
                        nc.tensor.matmul(ff2[:], f1[hh][:, bass.ts(jj, 128)],
                                         w2T[hh][:], start=(hh == 0),
                                         stop=(hh == HID // 128 - 1))
                    ot = slot.tile([128, DIM], F32, tag=f"ot{jj}")
                    nc.vector.tensor_tensor(ot[:], ff2[:], h_resN[jj][:], op=OP.add)
                    nc.sync.dma_start(out_r[p, jj], ot[:])
            return emit_tail

        # cross-chunk software pipeline: the first PEEL score units of each
        # chunk are emitted before the PREVIOUS chunk's tail, so PE has score
        # work covering the reciprocal/h_nrm latency; AV then runs PEEL
        # units behind scores for the rest of the chunk.
        PEEL = 4
        prev_tail = None
        for p in range(NCHUNK - 1, -1, -1):
            xqT = [xfT[p][i][:, 0:QC] for i in range(4)]
            hTA = psH.tile([128, 2 * QC], F32, tag="hTA", name="hTA")
            hTB = psH.tile([128, 2 * QC], F32, tag="hTB", name="hTB")
            hT = [hTA[:, bass.ts(0, QC)], hTA[:, bass.ts(1, QC)],
                  hTB[:, bass.ts(0, QC)], hTB[:, bass.ts(1, QC)]]
            s_rep = psR.tile([128, QC], F32, tag="srep")
            units = [(g, j) for g in range(p + 1) for j in range(4)]
            nu = len(units)
            emit_scores = make_scores(p, xqT)

            def emit_av(u, pt):
                g, j = units[u]
                first_av = (u == 0)
                last_av = (u == nu - 1)
                for i in range(4):
                    # start=True clears the whole PSUM bank, so only the
                    # first chain per packed bank may set it; the second
                    # chain writes into cleared has_written bits.
                    nc.tensor.matmul(hT[i], vc[g][j][:, bass.ts(i, 128)], pt[:],
                                     start=first_av and (i % 2 == 0),
                                     stop=last_av, skip_group_check=True)
                nc.tensor.matmul(s_rep[:], ones[:], pt[:],
                                 start=first_av, stop=last_av)

            S = min(PEEL, nu)
            pts = [None] * nu
            for u in range(S):
                pts[u] = emit_scores(*units[u])
            if prev_tail is not None:
                prev_tail()
            for u in range(S, nu):
                pts[u] = emit_scores(*units[u])
                emit_av(u - S, pts[u - S])
                pts[u - S] = None
                if p == NCHUNK - 1 and u % 2 == 0:
                    # one w1 block per TWO units: matches the ~2.3us/block
                    # DMA pace so PE never waits at the block transposes
                    next(gen1, None)
            for u in range(nu - S, nu):
                emit_av(u, pts[u])
                pts[u] = None
                if p == NCHUNK - 1 and u % 2 == 0:
                    next(gen1, None)
            prev_tail = make_tail(p, xqT, hT, s_rep)
        prev_tail()
        for _ in gen1:
            pass

    nc.compile()
    return nc


def _shard_inputs(inputs):
    x = np.ascontiguousarray(inputs["x"], dtype=np.float32)
    kr = np.arange(128)[:, None]
    qr = np.arange(QC)[None, :]
    mA = (qr >= kr).astype(np.float32)
    mB = (qr >= 128 + kr).astype(np.float32)
    in_maps = []
    for c in range(8):
        b, h = c // 2, c % 2
        xb = x[b]
        if h:
            # roll each 512-row group by 256 so this core's query rows are
            # always rolled rows [0,256) of the group
            xb = np.ascontiguousarray(
                xb.reshape(NCHUNK, 2, 256, DIM)[:, ::-1].reshape(SLEN, DIM))
        cmsk = np.stack([mA, mB, np.full_like(mA, h), np.full_like(mA, h)])
        m = {"xf": np.ascontiguousarray(xb), "cmask": np.ascontiguousarray(cmsk)}
        for k, v in inputs.items():
            if k not in ("x", "mask"):
                m[k] = np.ascontiguousarray(v, dtype=np.float32)
        in_maps.append(m)
    return in_maps


def kernel(**inputs):
    if "nc" not in _CACHE:
        _CACHE["nc"] = _build_nc()
    nc = _CACHE["nc"]
    in_maps = _shard_inputs(inputs)
    res = run_bass_kernel_spmd(nc, in_maps, core_ids=list(range(8)))
    out = np.empty((BS, SLEN, DIM), dtype=np.float32)
    for c in range(8):
        b, h = c // 2, c % 2
        o = res.results[c]["out"].reshape(NCHUNK, QC, DIM)
        out.reshape(BS, NCHUNK, 512, DIM)[b, :, QC * h:QC * h + QC, :] = o
    return out
